# revision 8
# baseline (speedup 1.0000x reference)
"""Distributed 3-layer GCN (N=100k nodes, E=800k edges, D=64) on 8 TRN2 NeuronCores.

Strategy (matches the sharding hint):
- Nodes are sharded across the 8 cores by destination octant (12500 nodes/core);
  edges are partitioned by destination node.
- Per layer, each core gathers source-node feature rows from a replicated
  node-major DRAM table via `dma_gather` (the halo exchange: the table for
  layers 2/3 is produced by an AllGather of the per-core shards; for layer 1
  the table is x itself, replicated at input-upload time).
- The segment-sum (scatter-add over destinations) is done with onehot matmuls:
  for each 128-destination tile, PSUM accumulates  msg[128e,64].T @ S[128e,128d]
  where S = (iota == dst_local) * norm is built in one DVE tensor_scalar op.
- GCN linearity lets each layer's weight be applied outside the aggregation,
  so no transposes are needed anywhere: epilogues run feature-major [64, 128].

kernel(**inputs) takes the FULL unsharded inputs and returns the FULL output.
"""

import os
import sys

sys.path.insert(0, "/opt/trn_rl_repo")

import numpy as np

from concourse import bacc, bass, mybir, tile  # noqa: E402
from concourse import bass_utils  # noqa: E402

F32 = mybir.dt.float32
I16 = mybir.dt.int16
P = 128
EPS = 1e-5

# set by test.py to request an ntff profile; results stashed in LAST_RESULT
PROFILE = bool(int(os.environ.get("KERNEL_PROFILE", "0")))
LAST_RESULT = None


def _ensure_ntff_hook():
    """Make trace=True work in this container: provide antenv.axon_hooks with a
    ctypes-based NTFF profile hook, and keep artifacts local (no bucket upload)."""
    bass_utils.upload_artifacts = lambda tmpdir: "local://" + tmpdir
    try:
        from antenv.axon_hooks import get_axon_ntff_profile_hook  # noqa: F401
        return
    except ImportError:
        pass
    import types

    import antenv

    mod = types.ModuleType("antenv.axon_hooks")
    holder = {}
    mod.set_axon_ntff_profile_hook = lambda h: holder.__setitem__("h", h)
    mod.get_axon_ntff_profile_hook = lambda: holder.get("h")
    sys.modules["antenv.axon_hooks"] = mod
    antenv.axon_hooks = mod
    if "/root/.axon_site" not in sys.path:
        sys.path.append("/root/.axon_site")
    try:
        from trn_agent_boot.trn_boot import _ntff_profile_via_ctypes
        hook = _ntff_profile_via_ctypes("/opt/axon/libaxon_pjrt.so")
        if hook is not None:
            mod.set_axon_ntff_profile_hook(hook)
    except Exception as e:  # profiling degrades, run still works
        print("ntff hook install failed:", e, file=sys.stderr)


class Cfg:
    def __init__(self, N=100000, E=800000, D=64, ncores=8, segsz=25000, group=14):
        assert N % ncores == 0
        self.N, self.E, self.D, self.ncores = N, E, D, ncores
        self.NL = N // ncores                       # nodes per core
        self.T = (self.NL + P - 1) // P             # dst tiles per core
        self.segsz = segsz                          # gather-table segment rows (int16 range)
        self.nseg = (N + segsz - 1) // segsz
        assert segsz <= 32768
        self.group = group                          # dst tiles per gather group


DEFAULT_CFG = Cfg()


def _prep(cfg, edge_index):
    """Host-side sharding prep. Returns per-core metadata arrays + layout tables."""
    N, ncores, NL, T, nseg, segsz = (
        cfg.N, cfg.ncores, cfg.NL, cfg.T, cfg.nseg, cfg.segsz)
    src = np.concatenate([edge_index[0], np.arange(N, dtype=np.int64)])
    dst = np.concatenate([edge_index[1], np.arange(N, dtype=np.int64)])
    deg = np.bincount(dst, minlength=N).astype(np.float32)
    dis = 1.0 / np.sqrt(deg)
    norm = (dis[src] * dis[dst]).astype(np.float32)

    cores = []
    counts = np.zeros((ncores, T, nseg), np.int64)
    for c in range(ncores):
        sel = (dst // NL) == c
        s_c = src[sel]
        dloc = (dst[sel] - c * NL).astype(np.int64)
        nm = norm[sel]
        tl = dloc // P
        sg = s_c // segsz
        order = np.lexsort((dloc, sg, tl))
        s_c, dloc, nm, tl, sg = (a[order] for a in (s_c, dloc, nm, tl, sg))
        np.add.at(counts[c], (tl, sg), 1)
        cores.append((s_c, dloc, nm, tl, sg))

    # uniform padded run lengths per (tile, seg): multiple of 128, same on all cores
    L = ((counts.max(axis=0) + P - 1) // P * P).astype(np.int64)   # [T, nseg]
    nidx = L.sum(axis=0)                                           # per segment
    segoff = np.zeros((T, nseg), np.int64)                         # run start per (t,s)
    segoff[1:] = np.cumsum(L, axis=0)[:-1]

    per_core = []
    for c in range(ncores):
        s_c, dloc, nm, tl, sg = cores[c]
        gidx = [np.zeros(nidx[s], np.int16) for s in range(nseg)]
        dl = [np.full(nidx[s], -1.0, np.float32) for s in range(nseg)]
        nmx = [np.zeros(nidx[s], np.float32) for s in range(nseg)]
        # edges are sorted by (tile, seg); place each run at its padded offset
        run_key = tl * nseg + sg
        boundaries = np.flatnonzero(np.diff(run_key)) + 1
        starts = np.concatenate([[0], boundaries])
        ends = np.concatenate([boundaries, [len(run_key)]])
        for a, b in zip(starts, ends):
            t, s = int(tl[a]), int(sg[a])
            o = segoff[t, s]
            n = b - a
            gidx[s][o:o + n] = (s_c[a:b] - s * segsz).astype(np.int16)
            dl[s][o:o + n] = (dloc[a:b] - t * P).astype(np.float32)
            nmx[s][o:o + n] = nm[a:b]
        # device layouts: idx i -> [i%16, i//16] (x8 partition replication);
        # edge i -> [i%128, i//128]
        g16 = [np.tile(g.reshape(-1, 16).T, (8, 1)) for g in gidx]
        dl128 = [d.reshape(-1, P).T.copy() for d in dl]
        nm128 = [d.reshape(-1, P).T.copy() for d in nmx]
        per_core.append((g16, dl128, nm128))
    return L, nidx, segoff, per_core


def _build(cfg, L, nidx, inp):
    """Build the SPMD bass program. L/nidx are uniform across cores."""
    N, D, T, NL, nseg, group = cfg.N, cfg.D, cfg.T, cfg.NL, cfg.nseg, cfg.group
    ncores = cfg.ncores
    nch = (L // P)                      # chunks per (t, s)
    segoff_ch = np.zeros((T, nseg), np.int64)
    segoff_ch[1:] = np.cumsum(nch, axis=0)[:-1]
    ngroups = (T + group - 1) // group
    # group boundaries in chunk units, per segment
    g_start = [[int(segoff_ch[g * group, s]) for s in range(nseg)]
               for g in range(ngroups)]
    g_end = [[int(segoff_ch[min((g + 1) * group, T) - 1, s] + nch[min((g + 1) * group, T) - 1, s])
              for s in range(nseg)] for g in range(ngroups)]
    mg = [max(g_end[g][s] - g_start[g][s] for g in range(ngroups)) for s in range(nseg)]

    dds = int(os.environ.get("KERNEL_DDS", "16384"))
    nc = bacc.Bacc(None, target_bir_lowering=False, debug=False, num_devices=ncores,
                   dynamic_dma_scratch_size=dds)

    xt = nc.dram_tensor("xt", [N, D], F32, kind="ExternalInput")
    gidx_d = [nc.dram_tensor(f"gidx{s}", [P, int(nidx[s]) // 16], I16, kind="ExternalInput")
              for s in range(nseg)]
    dl_d = [nc.dram_tensor(f"dl{s}", [P, int(nidx[s]) // P], F32, kind="ExternalInput")
            for s in range(nseg)]
    nm_d = [nc.dram_tensor(f"nm{s}", [P, int(nidx[s]) // P], F32, kind="ExternalInput")
            for s in range(nseg)]
    w_d = {k: nc.dram_tensor(k, [D, D], F32, kind="ExternalInput")
           for k in ("W1", "W2", "W3", "pW1", "pW2")}
    v_d = {k: nc.dram_tensor(k, [D, 1], F32, kind="ExternalInput")
           for k in ("scale1", "bias1", "scale2", "bias2", "mlpb")}
    pb2r = nc.dram_tensor("pb2r", [P, D], F32, kind="ExternalInput")
    iota_d = nc.dram_tensor("iota", [P, P], F32, kind="ExternalInput")
    out_d = nc.dram_tensor("out", [NL, D], F32, kind="ExternalOutput")

    tab_loc = [nc.dram_tensor(f"tabloc{i}", [NL, D], F32) for i in range(2)]
    tab_full = [nc.dram_tensor(f"tabfull{i}", [N, D], F32) for i in range(2)]

    with tile.TileContext(nc) as tc:
        with (
            tc.tile_pool(name="const", bufs=1) as cpool,
            tc.tile_pool(name="meta", bufs=1) as mpool,
            tc.tile_pool(name="msg0", bufs=2) as msg_pools0,
            tc.tile_pool(name="msg1", bufs=2) as msg_pools1,
            tc.tile_pool(name="msg2", bufs=2) as msg_pools2,
            tc.tile_pool(name="msg3", bufs=2) as msg_pools3,
            tc.tile_pool(name="sel", bufs=4) as selp,
            tc.tile_pool(name="epi", bufs=3) as epip,
            tc.tile_pool(name="ps", bufs=2, space="PSUM") as psp,
        ):
            msg_pools = [msg_pools0, msg_pools1, msg_pools2, msg_pools3][:nseg]
            # ---- load constants / metadata once ----
            iota_t = cpool.tile([P, P], F32)
            nc.sync.dma_start(iota_t[:], iota_d[:])
            wt = {}
            for k in w_d:
                wt[k] = cpool.tile([D, D], F32, name=f"w_{k}")
                nc.sync.dma_start(wt[k][:], w_d[k][:])
            vt = {}
            for k in v_d:
                vt[k] = cpool.tile([D, 1], F32, name=f"v_{k}")
                nc.sync.dma_start(vt[k][:], v_d[k][:])
            pb2r_t = cpool.tile([P, D], F32)
            nc.sync.dma_start(pb2r_t[:], pb2r[:])
            gidx_t, dl_t, nm_t = [], [], []
            for s in range(nseg):
                g = mpool.tile([P, int(nidx[s]) // 16], I16, name=f"gidx_sb{s}")
                nc.sync.dma_start(g[:], gidx_d[s][:])
                gidx_t.append(g)
                d = mpool.tile([P, int(nidx[s]) // P], F32, name=f"dl_sb{s}")
                nc.sync.dma_start(d[:], dl_d[s][:])
                dl_t.append(d)
                m = mpool.tile([P, int(nidx[s]) // P], F32, name=f"nm_sb{s}")
                nc.sync.dma_start(m[:], nm_d[s][:])
                nm_t.append(m)

            def run_layer(layer, table):
                for g in range(ngroups):
                    t0, t1 = g * group, min((g + 1) * group, T)
                    msg = []
                    for s in range(nseg):
                        cols = g_end[g][s] - g_start[g][s]
                        mt = msg_pools[s].tile([P, mg[s], D], F32, name=f"msg{s}")
                        if cols > 0:
                            n_i = cols * P
                            seg_rows = min(cfg.segsz, N - s * cfg.segsz)
                            nc.gpsimd.dma_gather(
                                out_ap=mt[:, :cols, :],
                                in_ap=table[s * cfg.segsz:
                                            s * cfg.segsz + seg_rows, :],
                                idxs_ap=gidx_t[s][:, g_start[g][s] * 8:
                                                  g_start[g][s] * 8 + n_i // 16],
                                num_idxs=n_i, num_idxs_reg=n_i, elem_size=D,
                                single_packet=bool(int(os.environ.get("KERNEL_SP", "0"))),
                            )
                        msg.append(mt)
                    for t in range(t0, t1):
                        nch_t = int(nch[t].sum())
                        agg = psp.tile([D, P], F32, name="aggps")
                        ci = 0
                        for s in range(nseg):
                            for j in range(int(nch[t, s])):
                                cg = int(segoff_ch[t, s]) + j
                                cl = cg - g_start[g][s]
                                sel = selp.tile([P, P], F32)
                                nc.vector.tensor_scalar(
                                    sel[:], iota_t[:],
                                    dl_t[s][:, cg:cg + 1], nm_t[s][:, cg:cg + 1],
                                    mybir.AluOpType.is_equal, mybir.AluOpType.mult)
                                nc.tensor.matmul(
                                    agg[:], msg[s][:, cl, :], sel[:],
                                    start=(ci == 0), stop=(ci == nch_t - 1))
                                ci += 1
                        rows = min(P, NL - t * P)
                        if layer == 0:
                            asb = epip.tile([D, P], F32)
                            nc.scalar.activation(asb[:], agg[:],
                                                 mybir.ActivationFunctionType.Copy)
                            h = psp.tile([D, P], F32, name="mm2ps")
                            nc.tensor.matmul(h[:], wt["W1"][:], asb[:],
                                             start=True, stop=True)
                            z = epip.tile([D, P], F32)
                            nc.scalar.activation(
                                z[:], h[:], mybir.ActivationFunctionType.Relu,
                                bias=vt["bias1"][:], scale=vt["scale1"][:])
                            nxt = psp.tile([P, D], F32, name="mm3ps")
                            nc.tensor.matmul(nxt[:], z[:], wt["W2"][:],
                                             start=True, stop=True)
                            hw = epip.tile([P, D], F32)
                            nc.scalar.activation(hw[:], nxt[:],
                                                 mybir.ActivationFunctionType.Copy)
                            nc.sync.dma_start(
                                tab_loc[0][t * P: t * P + rows, :], hw[:rows, :])
                        elif layer == 1:
                            z = epip.tile([D, P], F32)
                            nc.scalar.activation(
                                z[:], agg[:], mybir.ActivationFunctionType.Relu,
                                bias=vt["bias2"][:], scale=vt["scale2"][:])
                            nxt = psp.tile([P, D], F32, name="mm3ps")
                            nc.tensor.matmul(nxt[:], z[:], wt["W3"][:],
                                             start=True, stop=True)
                            hw = epip.tile([P, D], F32)
                            nc.scalar.activation(hw[:], nxt[:],
                                                 mybir.ActivationFunctionType.Copy)
                            nc.sync.dma_start(
                                tab_loc[1][t * P: t * P + rows, :], hw[:rows, :])
                        else:
                            asb = epip.tile([D, P], F32)
                            nc.scalar.activation(asb[:], agg[:],
                                                 mybir.ActivationFunctionType.Copy)
                            u = psp.tile([D, P], F32, name="mm2ps")
                            nc.tensor.matmul(u[:], wt["pW1"][:], asb[:],
                                             start=True, stop=True)
                            tt = epip.tile([D, P], F32)
                            nc.scalar.activation(
                                tt[:], u[:], mybir.ActivationFunctionType.Tanh,
                                bias=vt["mlpb"][:])
                            o = psp.tile([P, D], F32, name="mm3ps")
                            nc.tensor.matmul(o[:], tt[:], wt["pW2"][:],
                                             start=True, stop=True)
                            osb = epip.tile([P, D], F32)
                            nc.vector.tensor_tensor(osb[:], o[:], pb2r_t[:],
                                                    op=mybir.AluOpType.add)
                            nc.sync.dma_start(
                                out_d[t * P: t * P + rows, :], osb[:rows, :])

            run_layer(0, xt)
            nc.gpsimd.collective_compute(
                "AllGather", mybir.AluOpType.bypass,
                replica_groups=[list(range(ncores))],
                ins=[tab_loc[0].ap().opt()], outs=[tab_full[0].ap().opt()])
            run_layer(1, tab_full[0])
            nc.gpsimd.collective_compute(
                "AllGather", mybir.AluOpType.bypass,
                replica_groups=[list(range(ncores))],
                ins=[tab_loc[1].ap().opt()], outs=[tab_full[1].ap().opt()])
            run_layer(2, tab_full[1])
    nc.compile()
    return nc


def _host_consts(cfg, inp):
    D = cfg.D
    g1 = inp["bn1_g"] / np.sqrt(inp["bn1_v"] + EPS)
    g2 = inp["bn2_g"] / np.sqrt(inp["bn2_v"] + EPS)
    consts = {
        "W1": inp["W1"], "W2": inp["W2"], "W3": inp["W3"],
        "pW1": inp["pW1"], "pW2": inp["pW2"],
        "scale1": g1.reshape(D, 1),
        "bias1": ((inp["b1"] - inp["bn1_m"]) * g1 + inp["bn1_b"]).reshape(D, 1),
        "scale2": g2.reshape(D, 1),
        "bias2": ((inp["b2"] - inp["bn2_m"]) * g2 + inp["bn2_b"]).reshape(D, 1),
        "mlpb": (inp["pb1"] + inp["b3"] @ inp["pW1"]).reshape(D, 1),
        "pb2r": np.tile(inp["pb2"].reshape(1, D), (P, 1)),
        "iota": np.tile(np.arange(P, dtype=np.float32)[None, :], (P, 1)),
    }
    return {k: np.ascontiguousarray(v, dtype=np.float32) for k, v in consts.items()}


def make_in_maps(cfg, inp):
    edge_index = np.asarray(inp["edge_index"])
    L, nidx, segoff, per_core = _prep(cfg, edge_index)
    consts = _host_consts(cfg, inp)
    x = np.ascontiguousarray(np.asarray(inp["x"], dtype=np.float32))
    in_maps = []
    for c in range(cfg.ncores):
        g16, dl128, nm128 = per_core[c]
        m = {"xt": x, **consts}
        for s in range(cfg.nseg):
            m[f"gidx{s}"] = np.ascontiguousarray(g16[s])
            m[f"dl{s}"] = np.ascontiguousarray(dl128[s])
            m[f"nm{s}"] = np.ascontiguousarray(nm128[s])
        in_maps.append(m)
    return L, nidx, in_maps


def kernel(**inputs):
    global LAST_RESULT
    cfg = DEFAULT_CFG
    if PROFILE:
        _ensure_ntff_hook()
    L, nidx, in_maps = make_in_maps(cfg, inputs)
    nc = _build(cfg, L, nidx, inputs)
    res = bass_utils.run_bass_kernel_spmd(
        nc, in_maps, core_ids=list(range(cfg.ncores)), trace=PROFILE)
    LAST_RESULT = res
    out = np.concatenate([res.results[c]["out"] for c in range(cfg.ncores)], axis=0)
    return out
